# revision 5
# baseline (speedup 1.0000x reference)
"""KAN layer (identity edges) Trainium2 kernel.

output[b, o] = sum_i x[b, i]  for all o  -- row-sum broadcast to (B, 1024).

Data-parallel over 8 NeuronCores: each core gets 8192 rows of x
(65536 x 1024 f32), computes row sums on the Vector engine, broadcasts
them across the feature dim on-chip (cast to bf16, well within the 2e-2
tolerance), and DMAs the (8192, 1024) bf16 shard out; the host upcasts
back to f32.

Perf notes (HW-measured):
- bf16 store halves write traffic: 48 MiB/core total vs 64 MiB for f32
  (l2 rel err 1.7e-3, vs the 2e-2 gate).
- Tile = [128 partitions, 4 rows, 1024] so every load DMA descriptor is
  a 16 KiB contiguous per-partition chunk -- this keeps all 16 SDMA
  engines ~97% busy (~422 GB/s, the SBUF AXI port ceiling) when the
  sibling NeuronCore leaves HBM headroom; with full sibling overlap the
  per-stack HBM split (~716/2 GB/s) binds instead.
- Loads issue on the SP HWDGE ring, stores on the ACT ring, so the two
  streams feed the SDMA engines from separate queues.
- The last tile's reduce/cast/store chain is split in half so the
  end-of-stream serial tail is ~halved; the first half's cast runs on
  the Scalar (ACT) engine so it overlaps the second half's reduce on
  the Vector engine.

Layout: partition p owns 64 consecutive DRAM rows (rearrange
"(p n) d -> p n d"), so each DMA moves R*4KB contiguous bytes per
partition.
"""

import numpy as np

import concourse.tile as tile
from concourse import bacc, mybir
from concourse.bass_utils import run_bass_kernel_spmd

N_CORES = 8
BATCH = 65536
FEAT = 1024
ROWS = BATCH // N_CORES        # 8192 rows per core
P = 128                        # SBUF partitions
ROWS_PER_PART = ROWS // P      # 64 consecutive rows owned by each partition

R = 4                          # rows-per-partition per tile
IN_BUFS = 5
OUT_BUFS = 5

_nc_cache = []


def _build():
    n_iter = ROWS_PER_PART // R
    nc = bacc.Bacc()
    x = nc.declare_dram_parameter("x", [ROWS, FEAT], mybir.dt.float32, isOutput=False)
    y = nc.declare_dram_parameter("y", [ROWS, FEAT], mybir.dt.bfloat16, isOutput=True)
    xv = x[:, :].rearrange("(p n) d -> p n d", p=P)
    yv = y[:, :].rearrange("(p n) d -> p n d", p=P)

    with tile.TileContext(nc) as tc:
        with (
            tc.tile_pool(name="inp", bufs=IN_BUFS) as inp,
            tc.tile_pool(name="outp", bufs=OUT_BUFS) as outp,
            tc.tile_pool(name="sums", bufs=4) as sums_pool,
        ):
            for i in range(n_iter):
                t = inp.tile([P, R, FEAT], mybir.dt.float32)
                nc.sync.dma_start(out=t[:, :, :], in_=xv[:, i * R : (i + 1) * R, :])

                s = sums_pool.tile([P, R], mybir.dt.float32)
                o = outp.tile([P, R, FEAT], mybir.dt.bfloat16)
                # Split the last tile's reduce/cast/store into halves to
                # shorten the serial tail after the final load lands.
                k = 2 if i == n_iter - 1 else 1
                step = R // k
                for j in range(k):
                    a, b = j * step, (j + 1) * step
                    nc.vector.reduce_sum(
                        out=s[:, a:b], in_=t[:, a:b, :], axis=mybir.AxisListType.X
                    )
                    src = s[:, a:b].to_broadcast([P, b - a, FEAT])
                    if k > 1 and j == 0:
                        nc.scalar.copy(o[:, a:b, :], src)
                    else:
                        nc.vector.tensor_copy(out=o[:, a:b, :], in_=src)
                    nc.scalar.dma_start(
                        out=yv[:, i * R + a : i * R + b, :], in_=o[:, a:b, :]
                    )
    nc.finalize()
    return nc


def _get_nc():
    if not _nc_cache:
        _nc_cache.append(_build())
    return _nc_cache[0]


def kernel(x: np.ndarray) -> np.ndarray:
    nc = _get_nc()
    x = np.ascontiguousarray(np.asarray(x), dtype=np.float32)
    shards = np.split(x, N_CORES, axis=0)
    in_maps = [{"x": s} for s in shards]
    res = run_bass_kernel_spmd(nc, in_maps, list(range(N_CORES)))
    out = np.concatenate([res.results[i]["y"] for i in range(N_CORES)], axis=0)
    return out.astype(np.float32)
